# revision 48
# baseline (speedup 1.0000x reference)
"""Trainium2 Bass kernel for nn_EnhancedGNN (3-layer GCN + mean-pool + FC).

Contract: kernel(**inputs) takes FULL unsharded numpy inputs and returns the
FULL [64, 1] float32 output. Work is dst-sharded over 8 NeuronCores; all
feature data on device is bf16 (fp32 PSUM accumulation).

Design (vs the previous one-hot fp32 version):
  - gcn_norm (deg/dinv) is host-precomputed edge preprocessing; per-layer
    tables are h*dinv, 64 bf16 cols inside 256B-gatherable rows.
  - Layer 1 aggregates host-pregathered (x*dinv)[src] streamed contiguously
    (no dma_gather at all); layers 2/3 dma_gather their tables with calls
    round-robined over 4 SWDGE queues (4 Q7 core pairs emit descriptors
    concurrently -> ~2.3x gather throughput).
  - Aggregation is feature-major: per 128-edge block, lhsT = gathered rows
    [128,64] bf16 (scaled by w via one batched stride-0-broadcast DVE mult
    per gather call), rhs = one-hot(dstrel) [128,128] bf16; the one-hots for
    a whole window come from a single tensor_tensor is_equal against a
    stride-0-broadcast MM-ordered dstrel row. PSUM accumulates [64,128] per
    dst window (one full bank per window, window-major block order); the
    self-loop lands via an identity matmul into the same group.
  - Epilogues stay feature-major (per-feature bias on ACT partitions,
    per-node dinv via a shipped broadcast table), with a single PE transpose
    per window only where the node-major table row must be written; the
    batch one-hots for pooling are built once for all windows.
  - Tables are split in 4 parts (each < 32768 rows = one int16 chunk); each
    part AllGathers as soon as its ~25 windows are done, overlapping the
    collective with remaining compute.
  - Mean-pool via one-hot(batch) matmul accumulated over all windows;
    final FC + tiny AllReduce.
"""

import math
import os
import sys
import types

import numpy as np

# ---------------------------------------------------------------- constants
N_NODES = 100000
F_IN = 16
N_GRAPHS = 64
P = 128
N_CORES = 8
W = 98                                # windows of 128 dst nodes per core
NPC = W * P                           # 12544 nodes per core
NODES_PAD = N_CORES * NPC             # 100352
WPPS = [25, 25, 24, 24]               # windows per table part (4 parts)
PSTART = [0, 25, 50, 74]              # first window of each part
PART_ROWS = [w * P * N_CORES for w in WPPS]   # all < 32768 (int16 range)
N_CHUNKS = 4                          # chunk == part
GROUP_W = 4                           # windows per compute group
MAX_CALL_BLOCKS = 64
FD = 64                               # table feature cols (bf16); row = 128 bf16 = 256B
PART_OF_W = np.repeat(np.arange(4), WPPS).astype(np.int64)

LAST_EXEC_TIME_NS = None
LAST_TRACE = None
LAST_RESULT = None


# ---------------------------------------------------------------- host prep
def _groups():
    gs = []
    for part in range(4):
        lo = PSTART[part]
        for i in range(0, WPPS[part], GROUP_W):
            gs.append((part, lo + i, lo + min(i + GROUP_W, WPPS[part])))
    return gs


def _tpos(n):
    """node id -> (part, local table position within part)."""
    k = n // NPC
    r = n % NPC
    w = r // P
    p = r % P
    part = PART_OF_W[w]
    tl = (k * np.take(WPPS, part) + (w - np.take(PSTART, part))) * P + p
    return part, tl


def _prep(x, src, dst, w, batch, dinv):
    E = src.shape[0]
    core = dst // NPC
    wl = (dst % NPC) // P
    dstrel = dst % P

    part_s, tl = _tpos(src)
    ch = part_s
    idx16v = tl.astype(np.int16)

    key = (core * N_CHUNKS + ch) * W + wl
    cnt = np.bincount(key, minlength=N_CORES * N_CHUNKS * W)
    cnt = cnt.reshape(N_CORES, N_CHUNKS, W)
    nblk = -(-cnt.max(axis=0) // P)          # [N_CHUNKS, W], zero allowed
    for wloc in range(W):
        if nblk[:, wloc].sum() == 0:
            nblk[int(PART_OF_W[wloc]), wloc] = 1

    groups = _groups()
    blocks = []               # stream order: (g, ch, w, j)
    base_arr = np.zeros((N_CHUNKS, W), np.int64)
    calls = []                # (gidx, ch, b0, b1)
    group_brange = []         # (b0, b1) per group
    mm_blocks = []            # per group: list of (w, ch, bglob)
    for gidx, (part, wlo, whi) in enumerate(groups):
        gb0 = len(blocks)
        for c in range(N_CHUNKS):
            seg0 = len(blocks)
            for wloc in range(wlo, whi):
                n = int(nblk[c, wloc])
                base_arr[c, wloc] = len(blocks) * P
                for j in range(n):
                    blocks.append((c, wloc))
            seg1 = len(blocks)
            for b0 in range(seg0, seg1, MAX_CALL_BLOCKS):
                calls.append((gidx, c, b0, min(b0 + MAX_CALL_BLOCKS, seg1)))
        group_brange.append((gb0, len(blocks)))
        mm = []
        for wloc in range(wlo, whi):
            for c in range(N_CHUNKS):
                b = base_arr[c, wloc] // P
                for j in range(int(nblk[c, wloc])):
                    mm.append((wloc, c, b + j))
        mm_blocks.append(mm)
    NBLK = len(blocks)
    mm_order = [b for mm in mm_blocks for (_, _, b) in mm]
    bmaxw = int(nblk.sum(axis=0).max())
    NSLOT = NBLK * P
    nbmax = [1] * N_CHUNKS
    for (_, c, b0, b1) in calls:
        nbmax[c] = max(nbmax[c], b1 - b0)
    gbmax = max(b1 - b0 for (b0, b1) in group_brange)

    # per-edge slot position
    order = np.lexsort((wl, ch, core))
    skey = key[order]
    starts = np.flatnonzero(np.r_[True, skey[1:] != skey[:-1]])
    sizes = np.diff(np.r_[starts, E])
    rank = np.arange(E, dtype=np.int64) - np.repeat(starts, sizes)
    pos_sorted = base_arr[ch[order], wl[order]] + rank
    core_sorted = core[order]

    import ml_dtypes
    bf = ml_dtypes.bfloat16
    xd_pad = (x * dinv[:, None]).astype(np.float32)   # [NODES_PAD, 16]

    idx16 = np.zeros((N_CORES, NSLOT), np.int16)
    dstrel_s = np.zeros((N_CORES, NSLOT), np.float32)
    wslot = np.zeros((N_CORES, NSLOT), np.float32)
    xg = np.zeros((N_CORES, NSLOT, F_IN), np.float32)
    src_s = src[order]
    w_s = w[order]
    i16_s = idx16v[order]
    dr_s = dstrel[order]
    for k in range(N_CORES):
        m = core_sorted == k
        p = pos_sorted[m]
        idx16[k, p] = i16_s[m]
        dstrel_s[k, p] = dr_s[m]
        wslot[k, p] = w_s[m]
        xg[k, p, :] = w_s[m][:, None] * xd_pad[src_s[m]]

    idxw = np.tile(
        idx16.reshape(N_CORES, NSLOT // 16, 16).transpose(0, 2, 1), (1, 8, 1)
    )                                                  # [cores, 128, NSLOT/16]
    # dstrel in MM (window-major) block order, bf16, for batched C builds
    sdstm = (
        dstrel_s.reshape(N_CORES, NBLK, P)[:, mm_order, :]
        .transpose(0, 2, 1).astype(bf)
    )
    wst = wslot.reshape(N_CORES, NBLK, P).transpose(0, 2, 1).astype(bf)
    xgp = (
        xg.reshape(N_CORES, NBLK, P, F_IN)
        .transpose(0, 2, 1, 3)
        .astype(bf)
    )

    # per-core resident node data (inputs come pre-padded / pre-permuted)
    townf = np.zeros((N_CORES, FD, NPC), np.float32)
    dinvbc = np.zeros((N_CORES, FD, NPC), np.float32)
    batchf = batch.astype(np.float32)
    for k in range(N_CORES):
        sl = slice(k * NPC, (k + 1) * NPC)
        townf[k, :F_IN, :] = xd_pad[sl].T
        dinvbc[k, :, :] = dinv[sl][None, :]
    sbatch = batchf.reshape(N_CORES, W, P).transpose(0, 2, 1).astype(bf)

    meta = {
        "groups": groups, "calls": calls, "group_brange": group_brange,
        "mm_blocks": mm_blocks, "nblk": nblk, "NBLK": NBLK, "NSLOT": NSLOT,
        "nbmax": nbmax, "gbmax": gbmax, "bmaxw": bmaxw,
    }
    arrs = {
        "idxw": idxw, "sdstm": sdstm, "wst": wst, "xgp": xgp,
        "townf": townf.astype(bf), "dinvbc": dinvbc.astype(bf),
        "sbatch": sbatch,
    }
    return meta, arrs


# ------------------------------------------------------------- bass builder
def _build_nc(meta):
    import concourse.bacc as bacc
    import concourse.mybir as mybir
    import concourse.tile as tile
    from concourse.masks import make_identity

    f32 = mybir.dt.float32
    bf16 = mybir.dt.bfloat16
    i16 = mybir.dt.int16
    i32 = mybir.dt.int32
    AF = mybir.ActivationFunctionType
    OP = mybir.AluOpType

    groups = meta["groups"]
    calls = meta["calls"]
    group_brange = meta["group_brange"]
    mm_blocks = meta["mm_blocks"]
    NBLK = meta["NBLK"]
    NSLOT = meta["NSLOT"]
    nbmax = meta["nbmax"]
    gbmax = meta["gbmax"]
    bmaxw = meta["bmaxw"]

    nc = bacc.Bacc("TRN2", target_bir_lowering=False, debug=False,
                   num_devices=N_CORES, num_swdge_queues=4)

    idx_t = nc.dram_tensor("idxw", [P, NSLOT // 16], i16, kind="ExternalInput")
    dst_t = nc.dram_tensor("sdstm", [P, NBLK], bf16, kind="ExternalInput")
    wst_t = nc.dram_tensor("wst", [P, NBLK], bf16, kind="ExternalInput")
    xgp_t = nc.dram_tensor("xgp", [P, NBLK, F_IN], bf16,
                           kind="ExternalInput")
    townf_t = nc.dram_tensor("townf", [FD, NPC], bf16, kind="ExternalInput")
    dinvbc_t = nc.dram_tensor("dinvbc", [FD, NPC], bf16, kind="ExternalInput")
    batch_t = nc.dram_tensor("sbatch", [P, W], bf16, kind="ExternalInput")
    rcnt_t = nc.dram_tensor("rcntbc", [64, 64], f32, kind="ExternalInput")
    W1_t = nc.dram_tensor("W1b", [F_IN, 64], bf16, kind="ExternalInput")
    W2_t = nc.dram_tensor("W2b", [64, 128], bf16, kind="ExternalInput")
    W3_t = nc.dram_tensor("W3b", [128, 64], bf16, kind="ExternalInput")
    Wfc_t = nc.dram_tensor("Wfcb", [64, 1], bf16, kind="ExternalInput")
    b1_t = nc.dram_tensor("b1c", [64, 1], f32, kind="ExternalInput")
    b2_t = nc.dram_tensor("b2c", [128, 1], f32, kind="ExternalInput")
    b3_t = nc.dram_tensor("b3c", [64, 1], f32, kind="ExternalInput")
    bfc_t = nc.dram_tensor("bfcc", [64, 1], f32, kind="ExternalInput")
    out_t = nc.dram_tensor("out", [64, 1], f32, kind="ExternalOutput")

    RG = [list(range(N_CORES))]

    with tile.TileContext(nc) as tc:
        with (
            tc.tile_pool(name="dram", bufs=1, space="DRAM") as dram,
            tc.tile_pool(name="const", bufs=1) as const,
            tc.tile_pool(name="cmat", bufs=3) as cpool,
            tc.tile_pool(name="gat", bufs=2) as gpool,
            tc.tile_pool(name="gw", bufs=2) as gwpool,
            tc.tile_pool(name="xs", bufs=2) as xpool,
            tc.tile_pool(name="epi", bufs=3) as epool,
            tc.tile_pool(name="zps", bufs=3, space="PSUM") as zpool,
            tc.tile_pool(name="eps", bufs=2, space="PSUM") as espool,
            tc.tile_pool(name="pps", bufs=1, space="PSUM") as ppool,
        ):
            # DRAM: per-part tables + ag staging
            T = {}      # (layer, part) -> full table part
            AGT = {}    # (layer, part) -> own contribution
            for lyr in (2, 3):
                for part in range(4):
                    T[(lyr, part)] = dram.tile(
                        [PART_ROWS[part], 128], bf16, addr_space="Shared",
                        name=f"T{lyr}p{part}")
                    AGT[(lyr, part)] = dram.tile(
                        [WPPS[part] * P, 128], bf16, name=f"ag{lyr}p{part}")
            poolin = dram.tile([64, 1], f32)
            poolred = dram.tile([64, 1], f32, addr_space="Shared")

            # resident constants
            sid = const.tile([P, NSLOT // 16], i16)
            nc.sync.dma_start(out=sid[:], in_=idx_t[:])
            sdstm = const.tile([P, NBLK], bf16)
            nc.sync.dma_start(out=sdstm[:], in_=dst_t[:])
            swst = const.tile([P, NBLK], bf16)
            nc.sync.dma_start(out=swst[:], in_=wst_t[:])
            stownf = const.tile([FD, NPC], bf16)
            nc.sync.dma_start(out=stownf[:], in_=townf_t[:])
            sdinvbc = const.tile([FD, NPC], bf16)
            nc.sync.dma_start(out=sdinvbc[:], in_=dinvbc_t[:])
            sbatch = const.tile([P, W], bf16)
            nc.sync.dma_start(out=sbatch[:], in_=batch_t[:])
            srcnt = const.tile([64, 64], f32)
            nc.sync.dma_start(out=srcnt[:], in_=rcnt_t[:])
            sW1 = const.tile([F_IN, 64], bf16)
            nc.sync.dma_start(out=sW1[:], in_=W1_t[:])
            sW2 = const.tile([64, 128], bf16)
            nc.sync.dma_start(out=sW2[:], in_=W2_t[:])
            sW3 = const.tile([128, 64], bf16)
            nc.sync.dma_start(out=sW3[:], in_=W3_t[:])
            sWfc = const.tile([64, 1], bf16)
            nc.sync.dma_start(out=sWfc[:], in_=Wfc_t[:])
            sb1 = const.tile([64, 1], f32)
            nc.sync.dma_start(out=sb1[:], in_=b1_t[:])
            sb2 = const.tile([128, 1], f32)
            nc.sync.dma_start(out=sb2[:], in_=b2_t[:])
            sb3 = const.tile([64, 1], f32)
            nc.sync.dma_start(out=sb3[:], in_=b3_t[:])
            sbfc = const.tile([64, 1], f32)
            nc.sync.dma_start(out=sbfc[:], in_=bfc_t[:])

            iota_i = const.tile([P, P], i32)
            nc.gpsimd.iota(iota_i[:], pattern=[[1, P]], channel_multiplier=0)
            iota_b = const.tile([P, P], bf16)
            nc.vector.tensor_copy(out=iota_b[:], in_=iota_i[:])
            iota_rep = const.tile([P, bmaxw, P], bf16)
            for j in range(bmaxw):
                nc.vector.tensor_copy(out=iota_rep[:, j, :], in_=iota_b[:])
            iog_i = const.tile([P, 64], i32)
            nc.gpsimd.iota(iog_i[:], pattern=[[1, 64]], channel_multiplier=0)
            iog_b = const.tile([P, 64], bf16)
            nc.vector.tensor_copy(out=iog_b[:], in_=iog_i[:])
            identb = const.tile([P, P], bf16)
            make_identity(nc, identb[:])
            S_all = const.tile([P, W, 64], bf16)
            nc.vector.tensor_tensor(
                out=S_all[:, :, :],
                in0=iog_b[:].unsqueeze(1).broadcast_to([P, W, 64]),
                in1=sbatch[:].unsqueeze(2).broadcast_to([P, W, 64]),
                op=OP.is_equal,
            )
            stageA = const.tile([P, P], bf16)
            stageB = const.tile([P, P], bf16)
            stages = [stageA, stageB]

            pool_ps = ppool.tile([P, 512], f32, tag="pool")

            qcnt = [0]

            def chunk_src(lyr, c):
                return T[(lyr, c)][:, :]

            def emit_layer(lyr, epilogue):
                """lyr: 1 (xgp stream) or 2/3 (gathers)."""
                mmpos = 0
                for gidx, (part, wlo, whi) in enumerate(groups):
                    gb0, gb1 = group_brange[gidx]
                    nbg = gb1 - gb0
                    gtiles = {}
                    if lyr == 1:
                        xs = xpool.tile([P, gbmax, F_IN], bf16, tag="xs")
                        nc.sync.dma_start(
                            out=xs[:, 0:nbg, :],
                            in_=xgp_t[:, gb0:gb1, :])
                    else:
                        for (cg, c, b0, b1) in calls:
                            if cg != gidx:
                                continue
                            nb = b1 - b0
                            gt = gpool.tile([P, nbmax[c], 128], bf16,
                                            tag=f"g{c}")
                            nc.gpsimd.dma_gather(
                                out_ap=gt[:, :nb, :],
                                in_ap=chunk_src(lyr, c),
                                idxs_ap=sid[:, b0 * 8:b1 * 8],
                                num_idxs=nb * P, num_idxs_reg=nb * P,
                                elem_size=128, single_packet=False,
                                queue_num=(gidx + c) % 4,
                            )
                            gw = gwpool.tile([P, nbmax[c], FD], bf16,
                                             tag=f"w{c}")
                            nc.vector.tensor_tensor(
                                out=gw[:, :nb, :], in0=gt[:, :nb, 0:FD],
                                in1=swst[:, b0:b1].unsqueeze(2).broadcast_to(
                                    [P, nb, FD]),
                                op=OP.mult)
                            gtiles.setdefault(c, []).append((b0, b1, gw))
                    M = F_IN if lyr == 1 else FD
                    for wloc in range(wlo, whi):
                        sl = slice(wloc * P, (wloc + 1) * P)
                        zt = zpool.tile([P, 512], f32, tag="z")
                        blist = [mb for mb in mm_blocks[gidx] if mb[0] == wloc]
                        nw = len(blist)
                        Cw = cpool.tile([P, bmaxw, P], bf16, tag="Cw")
                        nc.vector.tensor_tensor(
                            out=Cw[:, 0:nw, :], in0=iota_rep[:, 0:nw, :],
                            in1=sdstm[:, mmpos:mmpos + nw].unsqueeze(
                                2).broadcast_to([P, nw, P]),
                            op=OP.is_equal,
                        )
                        for bi, (_, c, b) in enumerate(blist):
                            if lyr == 1:
                                lhsT = xs[:, b - gb0, :]
                            else:
                                for (b0, b1, gw) in gtiles[c]:
                                    if b0 <= b < b1:
                                        lhsT = gw[:, b - b0, 0:FD]
                                        break
                            nc.tensor.matmul(
                                out=zt[0:M, 0:128], lhsT=lhsT,
                                rhs=Cw[:, bi, :],
                                start=(bi == 0), stop=False,
                                skip_group_check=True,
                            )
                        mmpos += nw
                        # self-loop: z += TownF window slice (identity matmul)
                        nc.tensor.matmul(
                            out=zt[0:M, 0:128], lhsT=identb[0:M, 0:M],
                            rhs=stownf[0:M, sl], start=False, stop=True,
                            skip_group_check=True,
                        )
                        epilogue(wloc, part, zt)
                    if lyr < 3 and gidx in (6, 13, 19, 25):
                        nxt = lyr + 1
                        nc.gpsimd.collective_compute(
                            "AllGather", OP.bypass, replica_groups=RG,
                            ins=[AGT[(nxt, part)].opt()],
                            outs=[T[(nxt, part)].opt()],
                        )

            def write_table(lyr, wloc, part):
                """PE-transpose TownF slice -> node-major -> ag DRAM."""
                sl = slice(wloc * P, (wloc + 1) * P)
                wp = wloc - PSTART[part]
                tp = espool.tile([P, 1024], bf16, tag="tpb")
                nc.tensor.transpose(out=tp[:, 0:64], in_=stownf[:, sl],
                                    identity=identb[0:64, 0:64])
                stg = stages[wloc % 2]
                nc.scalar.activation(out=stg[:, 0:64], in_=tp[:, 0:64],
                                     func=AF.Copy)
                nc.sync.dma_start(
                    out=AGT[(lyr + 1, part)][wp * P:(wp + 1) * P, :],
                    in_=stg[:, :])

            def epi1(wloc, part, zt):
                sl = slice(wloc * P, (wloc + 1) * P)
                e2 = epool.tile([F_IN, P], bf16, tag="e2")
                nc.vector.tensor_tensor(out=e2[:], in0=zt[0:F_IN, 0:128],
                                        in1=sdinvbc[0:F_IN, sl], op=OP.mult)
                hp = espool.tile([P, 512], f32, tag="ep")
                nc.tensor.matmul(out=hp[0:64, 0:128], lhsT=sW1[:], rhs=e2[:],
                                 start=True, stop=True, skip_group_check=True)
                h1 = epool.tile([64, P], bf16, tag="h1")
                nc.scalar.activation(out=h1[:], in_=hp[0:64, 0:128],
                                     func=AF.Relu, bias=sb1[:])
                nc.vector.tensor_tensor(out=stownf[:, sl], in0=h1[:],
                                        in1=sdinvbc[:, sl], op=OP.mult)
                write_table(1, wloc, part)

            def epi2(wloc, part, zt):
                sl = slice(wloc * P, (wloc + 1) * P)
                e2 = epool.tile([FD, P], bf16, tag="e2f")
                nc.vector.tensor_tensor(out=e2[:], in0=zt[0:FD, 0:128],
                                        in1=sdinvbc[:, sl], op=OP.mult)
                hp = espool.tile([P, 512], f32, tag="ep")
                nc.tensor.matmul(out=hp[:, 0:128], lhsT=sW2[:], rhs=e2[:],
                                 start=True, stop=True, skip_group_check=True)
                h2 = epool.tile([P, P], bf16, tag="h2")
                nc.scalar.activation(out=h2[:], in_=hp[:, 0:128],
                                     func=AF.Relu, bias=sb2[:])
                tp3 = espool.tile([P, 512], f32, tag="ep")
                nc.tensor.matmul(out=tp3[0:64, 0:128], lhsT=sW3[:], rhs=h2[:],
                                 start=True, stop=True, skip_group_check=True)
                nc.vector.tensor_tensor(out=stownf[:, sl],
                                        in0=tp3[0:64, 0:128],
                                        in1=sdinvbc[:, sl], op=OP.mult)
                write_table(2, wloc, part)

            def epi3(wloc, part, zt):
                sl = slice(wloc * P, (wloc + 1) * P)
                e2 = epool.tile([FD, P], bf16, tag="e2f")
                nc.vector.tensor_tensor(out=e2[:], in0=zt[0:FD, 0:128],
                                        in1=sdinvbc[:, sl], op=OP.mult)
                h3 = epool.tile([FD, P], bf16, tag="h3")
                nc.scalar.activation(out=h3[:], in_=e2[:], func=AF.Relu,
                                     bias=sb3[:])
                tp = espool.tile([P, 1024], bf16, tag="tpb")
                nc.tensor.transpose(out=tp[:, 0:64], in_=h3[:],
                                    identity=identb[0:64, 0:64])
                h3nm = epool.tile([P, 64], bf16, tag="h3nm")
                nc.scalar.activation(out=h3nm[:], in_=tp[:, 0:64],
                                     func=AF.Copy)
                nc.tensor.matmul(
                    out=pool_ps[:64, 0:64], lhsT=h3nm[:],
                    rhs=S_all[:, wloc, :],
                    start=(wloc == 0), stop=(wloc == W - 1),
                    skip_group_check=True,
                )

            emit_layer(1, epi1)
            emit_layer(2, epi2)
            emit_layer(3, epi3)

            # ---- pooled [64 feat, 64 graph] -> mean -> FC -> AllReduce
            poolb = epool.tile([64, 64], bf16, tag="poolb")
            nc.vector.tensor_tensor(out=poolb[:], in0=pool_ps[:64, 0:64],
                                    in1=srcnt[:], op=OP.mult)
            op_ps = espool.tile([P, 512], f32, tag="ep")
            nc.tensor.matmul(out=op_ps[0:64, 0:1], lhsT=poolb[:], rhs=sWfc[:],
                             start=True, stop=True, skip_group_check=True)
            ocp = epool.tile([64, 1], f32, tag="ocp")
            nc.vector.tensor_copy(out=ocp[:], in_=op_ps[0:64, 0:1])
            nc.sync.dma_start(out=poolin[:], in_=ocp[:])
            nc.gpsimd.collective_compute(
                "AllReduce", OP.add, replica_groups=RG,
                ins=[poolin.opt()], outs=[poolred.opt()],
            )
            pr = epool.tile([64, 1], f32, tag="pr")
            nc.sync.dma_start(out=pr[:], in_=poolred[:])
            ob = epool.tile([64, 1], f32, tag="ob")
            nc.vector.tensor_tensor(out=ob[:], in0=pr[:], in1=sbfc[:],
                                    op=OP.add)
            nc.sync.dma_start(out=out_t[:], in_=ob[:])

    nc.finalize()
    return nc


# ------------------------------------------------------------------ runner
def _install_ntff_shim():
    try:
        import antenv
        if hasattr(antenv, "axon_hooks"):
            return
        mod = types.ModuleType("antenv.axon_hooks")
        mod._hook = None
        mod.set_axon_ntff_profile_hook = lambda h: setattr(mod, "_hook", h)
        mod.get_axon_ntff_profile_hook = lambda: mod._hook
        sys.modules["antenv.axon_hooks"] = mod
        antenv.axon_hooks = mod
        from trn_agent_boot.trn_boot import _ntff_profile_via_ctypes
        mod._hook = _ntff_profile_via_ctypes("/opt/axon/libaxon_pjrt.so")
    except Exception:
        pass


def kernel(x, edge_index, edge_weight, batch, W1, b1, W2, b2, W3, b3,
           Wfc, bfc):
    global LAST_EXEC_TIME_NS, LAST_TRACE, LAST_RESULT
    import ml_dtypes
    bf = ml_dtypes.bfloat16

    x = np.asarray(x, dtype=np.float32)
    ei = np.asarray(edge_index)
    src = ei[0].astype(np.int64)
    dst = ei[1].astype(np.int64)
    w = np.asarray(edge_weight, dtype=np.float32)
    batch = np.asarray(batch).astype(np.int64)

    # host gcn_norm preprocessing: deg = segsum(w, dst) + 1 (self loop)
    deg = np.bincount(dst, weights=w.astype(np.float64),
                      minlength=N_NODES).astype(np.float32) + 1.0
    dinv = 1.0 / np.sqrt(deg)

    # load-balance: relabel each core's windows by in-degree rank so heavy
    # windows of different cores align (shrinks max-over-cores block counts)
    wcnt = np.bincount(dst // P, minlength=NODES_PAD // P)
    perm = np.empty(NODES_PAD, np.int64)
    ar = np.arange(NODES_PAD, dtype=np.int64)
    for k in range(N_CORES):
        r = np.empty(W, np.int64)
        r[np.argsort(-wcnt[k * W:(k + 1) * W], kind="stable")] = np.arange(W)
        sl = slice(k * NPC, (k + 1) * NPC)
        n = ar[sl]
        perm[sl] = k * NPC + r[(n % NPC) // P] * P + n % P

    xp = np.zeros((NODES_PAD, F_IN), np.float32)
    xp[perm[:N_NODES]] = x
    bp = np.full(NODES_PAD, -1, np.int64)
    bp[perm[:N_NODES]] = batch
    dp = np.ones(NODES_PAD, np.float32)
    dp[perm[:N_NODES]] = dinv
    src = perm[src]
    dst = perm[dst]

    meta, arrs = _prep(xp, src, dst, w, bp, dp)

    cnt = np.bincount(batch, minlength=N_GRAPHS).astype(np.float32)
    rcnt = 1.0 / np.maximum(cnt, 1.0)
    rcntbc = np.broadcast_to(rcnt[None, :], (64, 64)).astype(np.float32).copy()

    W1b = np.asarray(W1, np.float32).astype(bf)
    W2b = np.asarray(W2, np.float32).astype(bf)
    W3b = np.asarray(W3, np.float32).astype(bf)
    Wfcb = np.asarray(Wfc, np.float32).reshape(64, 1).astype(bf)
    b1c = np.asarray(b1, np.float32).reshape(64, 1)
    b2c = np.asarray(b2, np.float32).reshape(128, 1)
    b3c = np.asarray(b3, np.float32).reshape(64, 1)
    bfcc = np.tile(np.asarray(bfc, np.float32).reshape(1, 1), (64, 1))

    nc = _build_nc(meta)

    in_maps = []
    for k in range(N_CORES):
        in_maps.append({
            "idxw": arrs["idxw"][k], "sdstm": arrs["sdstm"][k],
            "wst": arrs["wst"][k], "xgp": arrs["xgp"][k],
            "townf": arrs["townf"][k], "dinvbc": arrs["dinvbc"][k],
            "sbatch": arrs["sbatch"][k], "rcntbc": rcntbc,
            "W1b": W1b, "W2b": W2b, "W3b": W3b, "Wfcb": Wfcb,
            "b1c": b1c, "b2c": b2c, "b3c": b3c, "bfcc": bfcc,
        })

    trace = os.environ.get("BASS_GNN_TRACE", "") == "1"
    if trace:
        _install_ntff_shim()
        from concourse import bass_utils as _bu
        _bu.upload_artifacts = lambda tmpdir: tmpdir

    from concourse.bass_utils import run_bass_kernel_spmd
    res = run_bass_kernel_spmd(
        nc, in_maps, core_ids=list(range(N_CORES)), trace=trace,
    )
    LAST_RESULT = res
    if trace:
        LAST_EXEC_TIME_NS = res.exec_time_ns
        LAST_TRACE = (res.instructions_and_trace[1]
                      if res.instructions_and_trace else None)
    return np.asarray(res.results[0]["out"], dtype=np.float32)
